# revision 7
# baseline (speedup 1.0000x reference)
"""Contrastive (NT-Xent) loss kernel for 8 Trainium2 NeuronCores.

Math (reference): z = l2norm(concat(proj_1, proj_2)) [8192,128];
sim = z @ z.T; loss = mean_i( log(sum_{j!=i} exp(2*sim_ij)) - 2*pos_i ).

Sharding: rows of the 8192x8192 sim matrix are split 1024/core. Each core
receives the full rep matrix rotated by core*1024 rows (host-side layout
only), so its own rows are always local columns [0,1024) and the positive
partners are at [4096,5120) -- one identical SPMD program, static offsets.
Each core emits one partial scalar; the host sums 8 floats.

Device schedule (per core). ACT exp (1 elem/cycle/lane) is the bottleneck:
65536 elem/lane = 54.6us minimum, so everything is organized to start the
exp stream as early as possible and never stall it:
  - 16 chunk DMAs (512 rows, natural layout, bf16) all on the SP queue so
    the ACT sequencer never does DMA descriptor work.
  - per-chunk prep: 4x fused square+row-sum (DVE) -> 1/norm via ln/exp
    (compact, ACT, high priority) -> per-row scale (DVE) -> 4 PE
    transposes into normalized bf16 X^T columns.
  - exp ladder: first two tiles are [128,1024] (m0/m1 x cols 0:1024, only
    chunks 0-1 needed -> first exp at ~5.5us), then 30 tiles of
    [128,2048] (fused exp+row-sum into per-(m,tile) accum slots), with
    remaining chunk preps interleaved so DMA/DVE/PE work hides under the
    exp stream and PSUM slots rotate without stalling ACT.
  - finals: per-m rowsum reduces as each m finishes; one batched -E2 add,
    one Ln, partition-sum matmuls, single scalar DMA out.
"""

import ml_dtypes
import numpy as np

import concourse.bass as bass
import concourse.tile as tile
from concourse import bacc, mybir
from concourse.bass_utils import run_bass_kernel_spmd
from concourse.hw_specs import get_activation_tables
from concourse.masks import make_identity

B = 4096
D = 128
N2 = 2 * B            # 8192 total rows
NCORES = 8
RPC = N2 // NCORES    # 1024 rows per core
MT = RPC // 128       # 8 m-tiles of 128 rows
NCH = N2 // 512       # 16 column chunks of 512
TEMP = 0.5
E2 = float(np.exp(1.0 / TEMP))   # exp(sim_ii / T) with sim_ii == 1
TPM = 6               # max accum slots per m-tile

F32 = mybir.dt.float32
BF16 = mybir.dt.bfloat16
AX = mybir.AxisListType
OP = mybir.AluOpType
AF = mybir.ActivationFunctionType

LAST_RESULT = None  # BassKernelResults of the most recent run (for test.py)


def _build_nc():
    nc = bacc.Bacc("TRN2", target_bir_lowering=False)
    xn_d = nc.declare_dram_parameter("xn", [N2, D], BF16, isOutput=False)
    out_d = nc.declare_dram_parameter("out", [1, 1], F32, isOutput=True)

    # Pre-place the one ACT table set that covers both Ln and Exp, so the
    # greedy per-func chooser never inserts mid-kernel table switches.
    table_names = list(get_activation_tables(nc.m.arch).keys())
    combined_id = table_names.index("natural_log_exp_and_others")

    with tile.TileContext(nc) as tc:
        with (
            tc.tile_pool(name="big", bufs=1) as big,
            tc.tile_pool(name="work", bufs=3) as work,
            tc.tile_pool(name="scr", bufs=3) as scr,
            tc.tile_pool(name="ps", bufs=2, space="PSUM") as ps,
        ):
            nc.scalar.add_instruction(mybir.InstLoadActFuncSet(
                name=nc.get_next_instruction_name(), ins=[], outs=[],
                act_func_set_id=combined_id))

            xn_all = big.tile([128, 64, 128], BF16, tag="xn")  # [p, j, d]: row j*128+p
            xhat = big.tile([D, N2], BF16, tag="xhat")         # normalized reps^T
            ns_c = big.tile([128, 64], F32, tag="ns")          # |row|^2 compact
            lnn = big.tile([128, 64], F32, tag="lnn")
            s_c = big.tile([128, 64], F32, tag="s")            # 1/|row| compact
            ones_col = big.tile([128, 1], F32, tag="ones_col")
            rs = big.tile([128, MT * TPM], F32, tag="rs")      # exp row-sums (m, t)
            den8 = big.tile([128, MT], F32, tag="den8")
            logden = big.tile([128, MT], F32, tag="logden")
            ident = big.tile([128, 128], BF16, tag="ident")
            pacc = big.tile([128, 1], F32, tag="pacc")

            nc.vector.memset(ones_col, 1.0)
            nc.vector.memset(rs, 0.0)
            make_identity(nc, ident[:])

            # All 16 chunk DMAs upfront on the SP queue, in consumption
            # order; transfers serialize at ~0.73us per chunk so chunk c
            # lands at roughly 2 + 0.73c us.
            for c in range(NCH):
                nc.sync.dma_start(
                    out=xn_all[:, c * 4:(c + 1) * 4, :],
                    in_=xn_d[c * 512:(c + 1) * 512, :].rearrange(
                        "(t p) d -> p t d", p=128
                    ),
                )

            def prep_norms(c):
                """Fused square + row-sum for one 512-row chunk."""
                for j in range(4):
                    jj = c * 4 + j
                    sqs = work.tile([128, 128], F32, tag="sqs")
                    blk = xn_all[:, jj, :]
                    nc.vector.scalar_tensor_tensor(
                        out=sqs, in0=blk, scalar=1.0, in1=blk,
                        op0=OP.mult, op1=OP.mult,
                        accum_out=ns_c[:, jj:jj + 1],
                    )

            def norm_chain(norm_slice):
                # 1/norm = exp(-0.5*ln(normsq)); same ACT table set as the
                # main exp stream. High priority: these tiny ops gate the
                # scale->transpose chain and must not queue behind the exp
                # stream.
                with tc.high_priority():
                    nc.scalar.activation(
                        out=lnn[:, norm_slice], in_=ns_c[:, norm_slice],
                        func=AF.Ln,
                    )
                    nc.scalar.activation(
                        out=s_c[:, norm_slice], in_=lnn[:, norm_slice],
                        func=AF.Exp, scale=-0.5,
                    )

            def prep_finish(c):
                """Per-row scale, PE transpose into xhat columns."""
                xsc = work.tile([128, 4, 128], BF16, tag="xsc")
                nc.vector.tensor_mul(
                    xsc,
                    xn_all[:, c * 4:(c + 1) * 4, :],
                    s_c[:, c * 4:(c + 1) * 4].broadcast_to([128, 4, 128]),
                )
                tp = ps.tile([128, 512], BF16, tag="ps")
                for j in range(4):
                    nc.tensor.transpose(
                        tp[:, j * 128:(j + 1) * 128], xsc[:, j, :], ident[:]
                    )
                nc.vector.tensor_copy(xhat[:, c * 512:(c + 1) * 512], tp[:])

            def prep(c, norm_slice):
                prep_norms(c)
                norm_chain(norm_slice)
                prep_finish(c)

            def npair(c):
                """Norms for chunks c, c+1 plus one Ln/Exp pair chain; the
                scale+transpose halves are emitted separately (prep_finish)
                so PSUM slot use interleaves with the exp stream."""
                prep_norms(c)
                prep_norms(c + 1)
                norm_chain(slice(4 * c, 4 * c + 8))

            slot_cnt = [0] * MT  # next accum slot per m

            def exp_tile(m, col0, width):
                """matmuls for xhat cols [col0, col0+width) against m-tile m,
                then one fused exp+row-sum ACT op."""
                pst = ps.tile([128, width], F32, tag="ps")
                lhsT = xhat[:, m * 128:(m + 1) * 128]
                for s4 in range(width // 512):
                    col = col0 + s4 * 512
                    nc.tensor.matmul(
                        pst[:, s4 * 512:(s4 + 1) * 512],
                        lhsT=lhsT,
                        rhs=xhat[:, col:col + 512],
                        start=True,
                        stop=True,
                    )
                sc = scr.tile([128, width], BF16, tag="scr")
                t = slot_cnt[m]
                slot_cnt[m] += 1
                nc.scalar.activation(
                    out=sc, in_=pst, func=AF.Exp, scale=1.0 / TEMP,
                    accum_out=rs[:, m * TPM + t:m * TPM + t + 1],
                )

            def final_m(m):
                """den partial for m-tile m once all its tiles are in."""
                nc.vector.tensor_reduce(
                    out=den8[:, m:m + 1],
                    in_=rs[:, m * TPM:(m + 1) * TPM],
                    axis=AX.X, op=OP.add,
                )

            # Head: m0..m3 lhsT live in chunk 0, m4..m7 in chunk 1, so a
            # 512-wide ladder starts the exp stream as soon as chunk 0 is
            # transposed. Chunk preps interleave with exp tiles so the two
            # PSUM slots alternate transpose/exp use without stalling ACT.
            prep(0, norm_slice=slice(0, 4))
            prep(1, norm_slice=slice(4, 8))
            exp_tile(0, 0, 512)
            exp_tile(1, 0, 512)
            prep(2, norm_slice=slice(8, 12))
            exp_tile(2, 0, 512)
            exp_tile(3, 0, 512)
            prep(3, norm_slice=slice(12, 16))
            exp_tile(4, 0, 1024)
            npair(4)
            prep_finish(4)
            exp_tile(5, 0, 1024)
            prep_finish(5)
            exp_tile(6, 0, 1024)
            npair(6)
            prep_finish(6)
            exp_tile(7, 0, 1024)
            prep_finish(7)
            exp_tile(0, 512, 1536)
            npair(8)
            prep_finish(8)
            exp_tile(1, 512, 1536)
            prep_finish(9)
            exp_tile(2, 512, 1536)
            npair(10)
            prep_finish(10)
            exp_tile(3, 512, 1536)
            prep_finish(11)
            exp_tile(4, 1024, 1024)
            npair(12)
            prep_finish(12)
            exp_tile(5, 1024, 1024)
            prep_finish(13)
            exp_tile(6, 1024, 1024)
            npair(14)
            prep_finish(14)
            exp_tile(7, 1024, 1024)
            prep_finish(15)

            # positives dot: own rows (cols 0:1024) vs partners (4096:5120)
            prod = work.tile([128, RPC], F32, tag="pprod")
            nc.vector.scalar_tensor_tensor(
                out=prod, in0=xhat[:, 0:RPC], scalar=1.0,
                in1=xhat[:, B:B + RPC], op0=OP.mult, op1=OP.mult,
                accum_out=pacc,
            )

            # steady state: three 2048-wide sweeps cover cols [2048:8192)
            for m in range(MT):
                exp_tile(m, 2048, 2048)
            for m in range(MT):
                exp_tile(m, 4096, 2048)
            for m in (4, 5, 6, 7, 3, 2, 1, 0):
                exp_tile(m, 6144, 2048)
                final_m(m)

            # ---- finals ----
            nc.vector.tensor_scalar_add(out=den8, in0=den8, scalar1=-E2)
            nc.scalar.activation(out=logden, in_=den8, func=AF.Ln)
            ldps = ps.tile([1, MT], F32, tag="ps")
            nc.tensor.matmul(ldps, lhsT=ones_col, rhs=logden, start=True, stop=True)
            pps = ps.tile([1, 1], F32, tag="ps")
            nc.tensor.matmul(pps, lhsT=ones_col, rhs=pacc, start=True, stop=True)

            l1 = big.tile([1, 1], F32, tag="l1")
            nc.vector.tensor_reduce(out=l1, in_=ldps, axis=AX.X, op=OP.add)
            t2 = big.tile([1, 1], F32, tag="t2")
            nc.vector.tensor_scalar_mul(out=t2, in0=pps, scalar1=-2.0)
            res = big.tile([1, 1], F32, tag="res")
            nc.vector.tensor_add(res, l1, t2)
            nc.vector.tensor_scalar_mul(out=res, in0=res, scalar1=1.0 / N2)
            nc.sync.dma_start(out=out_d[:, :], in_=res)

    nc.compile()
    return nc


_NC = None


def kernel(proj_1: np.ndarray, proj_2: np.ndarray) -> np.ndarray:
    global _NC, LAST_RESULT
    import os

    reps = np.concatenate(
        [np.asarray(proj_1, np.float32), np.asarray(proj_2, np.float32)], axis=0
    )
    assert reps.shape == (N2, D)

    in_maps = [
        {"xn": np.ascontiguousarray(np.roll(reps, -c * RPC, axis=0)).astype(ml_dtypes.bfloat16)}
        for c in range(NCORES)
    ]

    if _NC is None:
        _NC = _build_nc()

    trace = bool(os.environ.get("CONTRASTIVE_TRACE"))
    result = run_bass_kernel_spmd(
        _NC, in_maps, core_ids=list(range(NCORES)), trace=trace
    )
    LAST_RESULT = result
    total = sum(float(r["out"][0, 0]) for r in result.results)
    return np.float32(total)


# revision 14
# speedup vs baseline: 1.0135x; 1.0135x over previous
"""Contrastive (NT-Xent) loss kernel for 8 Trainium2 NeuronCores.

Math (reference): z = l2norm(concat(proj_1, proj_2)) [8192,128];
sim = z @ z.T; loss = mean_i( log(sum_{j!=i} exp(2*sim_ij)) - 2*pos_i ).

Sharding: rows of the 8192x8192 sim matrix are split 1024/core. Each core
receives the full rep matrix rotated by core*1024 rows (host-side layout
only), so its own rows are always local columns [0,1024) and the positive
partners are at [4096,5120) -- one identical SPMD program, static offsets.
Each core emits one partial scalar; the host sums 8 floats.

Device pipeline (per core), interleaved in groups of 2048 columns so the
ACT engine (the bottleneck: exp at 1 elem/cycle/lane) starts ~5us in:
  group g: DMA 4 natural bf16 chunks -> row norms (DVE, fp32 accum) ->
  1/norm via ln/exp (compact [128,16], ACT) -> per-row scale (DVE) ->
  PE-transpose into normalized bf16 X^T columns -> main quarter g:
  8 m-tiles x (4 bf16 matmuls -> fused exp+row-sum ACT op, [128,2048]
  PSUM). bf16 input halves the DMA head; loss rel err stays ~1e-6.
Then log-denominator, positives dot, partition-sum matmuls, one scalar out.
"""

import ml_dtypes
import numpy as np

import concourse.bass as bass
import concourse.tile as tile
from concourse import bacc, mybir
from concourse.bass_utils import run_bass_kernel_spmd
from concourse.hw_specs import get_activation_tables
from concourse.masks import make_identity

B = 4096
D = 128
N2 = 2 * B            # 8192 total rows
NCORES = 8
RPC = N2 // NCORES    # 1024 rows per core
MT = RPC // 128       # 8 m-tiles of 128 rows
NCH = N2 // 512       # 16 column chunks of 512
NG = 4                # groups of 4 chunks (2048 cols)
TEMP = 0.5
E2 = float(np.exp(1.0 / TEMP))   # exp(sim_ii / T) with sim_ii == 1

F32 = mybir.dt.float32
F32R = mybir.dt.float32r
BF16 = mybir.dt.bfloat16
AX = mybir.AxisListType
OP = mybir.AluOpType
AF = mybir.ActivationFunctionType

LAST_RESULT = None  # BassKernelResults of the most recent run (for test.py)


def _build_nc():
    nc = bacc.Bacc("TRN2", target_bir_lowering=False)
    xn_d = nc.declare_dram_parameter("xn", [N2, D], BF16, isOutput=False)
    out_d = nc.declare_dram_parameter("out", [1, 1], F32, isOutput=True)

    # Pre-place the one ACT table set that covers both Ln and Exp, so the
    # greedy per-func chooser never inserts mid-kernel table switches.
    table_names = list(get_activation_tables(nc.m.arch).keys())
    combined_id = table_names.index("natural_log_exp_and_others")

    with tile.TileContext(nc) as tc:
        with (
            tc.tile_pool(name="big", bufs=1) as big,
            tc.tile_pool(name="work", bufs=3) as work,
            tc.tile_pool(name="scr", bufs=2) as scr,
            tc.tile_pool(name="ps", bufs=2, space="PSUM") as ps,
        ):
            nc.scalar.add_instruction(mybir.InstLoadActFuncSet(
                name=nc.get_next_instruction_name(), ins=[], outs=[],
                act_func_set_id=combined_id))

            xn_all = big.tile([128, 64, 128], BF16, tag="xn")  # [p, j, d]: row j*128+p
            xhat = big.tile([D, N2], BF16, tag="xhat")         # normalized reps^T
            ns_c = big.tile([128, 64], F32, tag="ns")         # |row|^2 compact
            lnn = big.tile([128, 64], F32, tag="lnn")
            s_c = big.tile([128, 64], F32, tag="s")           # 1/|row| compact
            ones_col = big.tile([128, 1], F32, tag="ones_col")
            rs_all = big.tile([128, MT * NG], F32, tag="rs")  # exp row-sums (m, g)
            ident = big.tile([128, 128], BF16, tag="ident")
            pacc = big.tile([128, 1], F32, tag="pacc")

            def prep_group(g):
                """DMA 4 natural chunks, row norms, 1/norm, scale, transpose
                into xhat columns [2048g, 2048(g+1))."""
                for c in range(4 * g, 4 * g + 4):
                    eng = nc.sync
                    eng.dma_start(
                        out=xn_all[:, c * 4:(c + 1) * 4, :],
                        in_=xn_d[c * 512:(c + 1) * 512, :].rearrange(
                            "(t p) d -> p t d", p=128
                        ),
                    )
                    # normsq per 128-row block: fused square + row-sum
                    for j in range(4):
                        jj = c * 4 + j
                        sqs = work.tile([128, 128], F32, tag="sqs")
                        blk = xn_all[:, jj, :]
                        nc.vector.scalar_tensor_tensor(
                            out=sqs, in0=blk, scalar=1.0, in1=blk,
                            op0=OP.mult, op1=OP.mult,
                            accum_out=ns_c[:, jj:jj + 1],
                        )
                # 1/norm = exp(-0.5*ln(normsq)); same ACT table set as exp.
                # group 0 is the latency-critical head: do it per chunk-pair
                # so the chain doesn't wait for all 4 chunk DMAs.
                subs = 2 if g == 0 else 1
                # high priority: these tiny ops must not queue behind the
                # previous quarter's exp stream on ACT (they gate this
                # group's scale->transpose chain and its PSUM slot release)
                with tc.high_priority():
                    for i in range(subs):
                        w = 16 // subs
                        gsl = slice(16 * g + i * w, 16 * g + (i + 1) * w)
                        nc.scalar.activation(
                            out=lnn[:, gsl], in_=ns_c[:, gsl], func=AF.Ln
                        )
                        nc.scalar.activation(
                            out=s_c[:, gsl], in_=lnn[:, gsl], func=AF.Exp,
                            scale=-0.5,
                        )
                # scale rows, PE-transpose into xhat columns (bf16)
                tp = ps.tile([128, 2048], BF16, tag="ps")
                for c in range(4 * g, 4 * g + 4):
                    xsc = work.tile([128, 4, 128], BF16, tag="xsc")
                    nc.vector.tensor_mul(
                        xsc,
                        xn_all[:, c * 4:(c + 1) * 4, :],
                        s_c[:, c * 4:(c + 1) * 4].broadcast_to([128, 4, 128]),
                    )
                    for j in range(4):
                        nc.tensor.transpose(
                            tp[:, (c % 4) * 512 + j * 128:(c % 4) * 512 + (j + 1) * 128],
                            xsc[:, j, :],
                            ident[:],
                        )
                    nc.vector.tensor_copy(
                        xhat[:, c * 512:(c + 1) * 512],
                        tp[:, (c % 4) * 512:(c % 4 + 1) * 512],
                    )
                if g == 2:
                    # positives dot (needs xhat chunks 0,1 and 8,9)
                    prod = scr.tile([128, RPC], F32, tag="scr")
                    nc.vector.scalar_tensor_tensor(
                        out=prod,
                        in0=xhat[:, 0:RPC],
                        scalar=1.0,
                        in1=xhat[:, B:B + RPC],
                        op0=OP.mult,
                        op1=OP.mult,
                        accum_out=pacc,
                    )

            def quarter_half(g, half):
                """4 m-tiles of main work on columns [2048g, 2048(g+1))."""
                for m in range(4 * half, 4 * half + 4):
                    pst = ps.tile([128, 2048], F32, tag="ps")
                    lhsT = xhat[:, m * 128:(m + 1) * 128]
                    for s4 in range(4):
                        col = g * 2048 + s4 * 512
                        nc.tensor.matmul(
                            pst[:, s4 * 512:(s4 + 1) * 512],
                            lhsT=lhsT,
                            rhs=xhat[:, col:col + 512],
                            start=True,
                            stop=True,
                        )
                    sc = scr.tile([128, 2048], BF16, tag="scr")
                    nc.scalar.activation(
                        out=sc,
                        in_=pst,
                        func=AF.Exp,
                        scale=1.0 / TEMP,
                        accum_out=rs_all[:, m * NG + g:m * NG + g + 1],
                    )

            nc.vector.memset(ones_col, 1.0)
            make_identity(nc, ident[:])

            # interleave: group g+1 prep emitted mid-quarter-g so its DMAs,
            # DVE work and PSUM slot use hide under the ACT exp stream
            prep_group(0)
            quarter_half(0, 0)
            prep_group(1)
            quarter_half(0, 1)
            quarter_half(1, 0)
            prep_group(2)
            quarter_half(1, 1)
            quarter_half(2, 0)
            prep_group(3)
            quarter_half(2, 1)
            quarter_half(3, 0)

            # ---- finals, first half: m0-3 dens are complete now, so their
            # log chain runs under quarter (3,1)'s exp stream ----
            rowsum = big.tile([128, MT], F32, tag="rowsum")
            logden = big.tile([128, MT], F32, tag="logden")
            pps = ps.tile([1, 1], F32, tag="ps")
            nc.tensor.matmul(pps, lhsT=ones_col, rhs=pacc, start=True, stop=True)
            nc.vector.tensor_reduce(
                out=rowsum[:, 0:4],
                in_=rs_all[:, 0:4 * NG].rearrange("p (m g) -> p m g", g=NG),
                axis=AX.X,
                op=OP.add,
            )
            nc.vector.tensor_scalar_add(
                out=rowsum[:, 0:4], in0=rowsum[:, 0:4], scalar1=-E2)
            nc.scalar.activation(
                out=logden[:, 0:4], in_=rowsum[:, 0:4], func=AF.Ln)

            quarter_half(3, 1)

            # ---- finals, second half ----
            nc.vector.tensor_reduce(
                out=rowsum[:, 4:8],
                in_=rs_all[:, 4 * NG:8 * NG].rearrange("p (m g) -> p m g", g=NG),
                axis=AX.X,
                op=OP.add,
            )
            nc.vector.tensor_scalar_add(
                out=rowsum[:, 4:8], in0=rowsum[:, 4:8], scalar1=-E2)
            nc.scalar.activation(
                out=logden[:, 4:8], in_=rowsum[:, 4:8], func=AF.Ln)
            ldps = ps.tile([1, MT], F32, tag="ps")
            nc.tensor.matmul(ldps, lhsT=ones_col, rhs=logden, start=True, stop=True)

            l1 = big.tile([1, 1], F32, tag="l1")
            nc.vector.tensor_reduce(out=l1, in_=ldps, axis=AX.X, op=OP.add)
            res = big.tile([1, 1], F32, tag="res")
            # res = (l1 - 2*pps) / N2 in two fused ops
            nc.vector.scalar_tensor_tensor(
                out=res, in0=pps, scalar=-2.0, in1=l1,
                op0=OP.mult, op1=OP.add,
            )
            nc.vector.tensor_scalar_mul(out=res, in0=res, scalar1=1.0 / N2)
            nc.sync.dma_start(out=out_d[:, :], in_=res)

    nc.compile()
    return nc


_NC = None


def kernel(proj_1: np.ndarray, proj_2: np.ndarray) -> np.ndarray:
    global _NC, LAST_RESULT
    import os

    reps = np.concatenate(
        [np.asarray(proj_1, np.float32), np.asarray(proj_2, np.float32)], axis=0
    )
    assert reps.shape == (N2, D)

    in_maps = [
        {"xn": np.ascontiguousarray(np.roll(reps, -c * RPC, axis=0)).astype(ml_dtypes.bfloat16)}
        for c in range(NCORES)
    ]

    if _NC is None:
        _NC = _build_nc()

    trace = bool(os.environ.get("CONTRASTIVE_TRACE"))
    result = run_bass_kernel_spmd(
        _NC, in_maps, core_ids=list(range(NCORES)), trace=trace
    )
    LAST_RESULT = result
    total = sum(float(r["out"][0, 0]) for r in result.results)
    return np.float32(total)



# revision 15
# speedup vs baseline: 1.0299x; 1.0162x over previous
"""Contrastive (NT-Xent) loss kernel for 8 Trainium2 NeuronCores.

Math (reference): z = l2norm(concat(proj_1, proj_2)) [8192,128];
sim = z @ z.T; loss = mean_i( log(sum_{j!=i} exp(2*sim_ij)) - 2*pos_i ).

Sharding: rows of the 8192x8192 sim matrix are split 1024/core. Each core
receives the full rep matrix rotated by core*1024 rows (host-side layout
only), so its own rows are always local columns [0,1024) and the positive
partners are at [4096,5120) -- one identical SPMD program, static offsets.
Each core emits one partial scalar; the host sums 8 floats.

Device pipeline (per core), interleaved in groups of 2048 columns so the
ACT engine (the bottleneck: exp at 1 elem/cycle/lane) starts ~5us in:
  group g: DMA 4 natural bf16 chunks -> row norms (DVE, fp32 accum) ->
  1/norm via ln/exp (compact [128,16], ACT) -> per-row scale (DVE) ->
  PE-transpose into normalized bf16 X^T columns -> main quarter g:
  8 m-tiles x (4 bf16 matmuls -> fused exp+row-sum ACT op, [128,2048]
  PSUM). bf16 input halves the DMA head; loss rel err stays ~1e-6.
Then log-denominator, positives dot, partition-sum matmuls, one scalar out.
"""

import ml_dtypes
import numpy as np

import concourse.bass as bass
import concourse.tile as tile
from concourse import bacc, mybir
from concourse.bass_utils import run_bass_kernel_spmd
from concourse.hw_specs import get_activation_tables
from concourse.masks import make_identity

B = 4096
D = 128
N2 = 2 * B            # 8192 total rows
NCORES = 8
RPC = N2 // NCORES    # 1024 rows per core
MT = RPC // 128       # 8 m-tiles of 128 rows
NCH = N2 // 512       # 16 column chunks of 512
NG = 4                # groups of 4 chunks (2048 cols)
TEMP = 0.5
E2 = float(np.exp(1.0 / TEMP))   # exp(sim_ii / T) with sim_ii == 1

F32 = mybir.dt.float32
F32R = mybir.dt.float32r
BF16 = mybir.dt.bfloat16
AX = mybir.AxisListType
OP = mybir.AluOpType
AF = mybir.ActivationFunctionType

LAST_RESULT = None  # BassKernelResults of the most recent run (for test.py)


def _build_nc():
    nc = bacc.Bacc("TRN2", target_bir_lowering=False)
    xn_d = nc.declare_dram_parameter("xn", [N2, D], BF16, isOutput=False)
    out_d = nc.declare_dram_parameter("out", [1, 1], F32, isOutput=True)

    # Pre-place the one ACT table set that covers both Ln and Exp, so the
    # greedy per-func chooser never inserts mid-kernel table switches.
    table_names = list(get_activation_tables(nc.m.arch).keys())
    combined_id = table_names.index("natural_log_exp_and_others")

    with tile.TileContext(nc) as tc:
        with (
            tc.tile_pool(name="big", bufs=1) as big,
            tc.tile_pool(name="work", bufs=3) as work,
            tc.tile_pool(name="scr", bufs=2) as scr,
            tc.tile_pool(name="ps", bufs=2, space="PSUM") as ps,
        ):
            nc.scalar.add_instruction(mybir.InstLoadActFuncSet(
                name=nc.get_next_instruction_name(), ins=[], outs=[],
                act_func_set_id=combined_id))

            xn_all = big.tile([128, 64, 128], BF16, tag="xn")  # [p, j, d]: row j*128+p
            xhat = big.tile([D, N2], BF16, tag="xhat")         # normalized reps^T
            ns_c = big.tile([128, 64], F32, tag="ns")         # |row|^2 compact
            lnn = big.tile([128, 64], F32, tag="lnn")
            s_c = big.tile([128, 64], F32, tag="s")           # 1/|row| compact
            ones_col = big.tile([128, 1], F32, tag="ones_col")
            rs_all = big.tile([128, MT * NG], F32, tag="rs")  # exp row-sums (m, g)
            ident = big.tile([128, 128], BF16, tag="ident")
            pacc = big.tile([128, 1], F32, tag="pacc")

            def prep_group(g):
                """DMA 4 natural chunks, row norms, 1/norm, scale, transpose
                into xhat columns [2048g, 2048(g+1))."""
                for c in range(4 * g, 4 * g + 4):
                    # alternate the two HWDGE queues (SP / ACT)
                    eng = nc.sync if c % 2 == 0 else nc.scalar
                    eng.dma_start(
                        out=xn_all[:, c * 4:(c + 1) * 4, :],
                        in_=xn_d[c * 512:(c + 1) * 512, :].rearrange(
                            "(t p) d -> p t d", p=128
                        ),
                    )
                    # normsq per 128-row block: fused square + row-sum
                    for j in range(4):
                        jj = c * 4 + j
                        sqs = work.tile([128, 128], F32, tag="sqs")
                        blk = xn_all[:, jj, :]
                        nc.vector.scalar_tensor_tensor(
                            out=sqs, in0=blk, scalar=1.0, in1=blk,
                            op0=OP.mult, op1=OP.mult,
                            accum_out=ns_c[:, jj:jj + 1],
                        )
                # 1/norm = exp(-0.5*ln(normsq)); same ACT table set as exp.
                # group 0 is the latency-critical head: do it per chunk-pair
                # so the chain doesn't wait for all 4 chunk DMAs.
                subs = 2 if g == 0 else 1
                # high priority: these tiny ops must not queue behind the
                # previous quarter's exp stream on ACT (they gate this
                # group's scale->transpose chain and its PSUM slot release)
                with tc.high_priority():
                    for i in range(subs):
                        w = 16 // subs
                        gsl = slice(16 * g + i * w, 16 * g + (i + 1) * w)
                        nc.scalar.activation(
                            out=lnn[:, gsl], in_=ns_c[:, gsl], func=AF.Ln
                        )
                        nc.scalar.activation(
                            out=s_c[:, gsl], in_=lnn[:, gsl], func=AF.Exp,
                            scale=-0.5,
                        )
                # scale rows, PE-transpose into xhat columns (bf16)
                tp = ps.tile([128, 2048], BF16, tag="ps")
                for c in range(4 * g, 4 * g + 4):
                    xsc = work.tile([128, 4, 128], BF16, tag="xsc")
                    nc.vector.tensor_mul(
                        xsc,
                        xn_all[:, c * 4:(c + 1) * 4, :],
                        s_c[:, c * 4:(c + 1) * 4].broadcast_to([128, 4, 128]),
                    )
                    for j in range(4):
                        nc.tensor.transpose(
                            tp[:, (c % 4) * 512 + j * 128:(c % 4) * 512 + (j + 1) * 128],
                            xsc[:, j, :],
                            ident[:],
                        )
                    nc.vector.tensor_copy(
                        xhat[:, c * 512:(c + 1) * 512],
                        tp[:, (c % 4) * 512:(c % 4 + 1) * 512],
                    )
                if g == 2:
                    # positives dot (needs xhat chunks 0,1 and 8,9)
                    prod = scr.tile([128, RPC], F32, tag="scr")
                    nc.vector.scalar_tensor_tensor(
                        out=prod,
                        in0=xhat[:, 0:RPC],
                        scalar=1.0,
                        in1=xhat[:, B:B + RPC],
                        op0=OP.mult,
                        op1=OP.mult,
                        accum_out=pacc,
                    )

            def quarter_half(g, half):
                """4 m-tiles of main work on columns [2048g, 2048(g+1))."""
                for m in range(4 * half, 4 * half + 4):
                    pst = ps.tile([128, 2048], F32, tag="ps")
                    lhsT = xhat[:, m * 128:(m + 1) * 128]
                    for s4 in range(4):
                        col = g * 2048 + s4 * 512
                        nc.tensor.matmul(
                            pst[:, s4 * 512:(s4 + 1) * 512],
                            lhsT=lhsT,
                            rhs=xhat[:, col:col + 512],
                            start=True,
                            stop=True,
                        )
                    sc = scr.tile([128, 2048], BF16, tag="scr")
                    nc.scalar.activation(
                        out=sc,
                        in_=pst,
                        func=AF.Exp,
                        scale=1.0 / TEMP,
                        accum_out=rs_all[:, m * NG + g:m * NG + g + 1],
                    )

            nc.vector.memset(ones_col, 1.0)
            make_identity(nc, ident[:])

            # interleave: group g+1 prep emitted mid-quarter-g so its DMAs,
            # DVE work and PSUM slot use hide under the ACT exp stream
            prep_group(0)
            quarter_half(0, 0)
            prep_group(1)
            quarter_half(0, 1)
            quarter_half(1, 0)
            prep_group(2)
            quarter_half(1, 1)
            quarter_half(2, 0)
            prep_group(3)
            quarter_half(2, 1)
            quarter_half(3, 0)

            # ---- finals, first half: m0-3 dens are complete now, so their
            # log chain runs under quarter (3,1)'s exp stream ----
            rowsum = big.tile([128, MT], F32, tag="rowsum")
            logden = big.tile([128, MT], F32, tag="logden")
            nc.vector.tensor_reduce(
                out=rowsum[:, 0:4],
                in_=rs_all[:, 0:4 * NG].rearrange("p (m g) -> p m g", g=NG),
                axis=AX.X,
                op=OP.add,
            )
            nc.vector.tensor_scalar_add(
                out=rowsum[:, 0:4], in0=rowsum[:, 0:4], scalar1=-E2)
            nc.scalar.activation(
                out=logden[:, 0:4], in_=rowsum[:, 0:4], func=AF.Ln)

            quarter_half(3, 1)

            # ---- finals, second half ----
            nc.vector.tensor_reduce(
                out=rowsum[:, 4:8],
                in_=rs_all[:, 4 * NG:8 * NG].rearrange("p (m g) -> p m g", g=NG),
                axis=AX.X,
                op=OP.add,
            )
            nc.vector.tensor_scalar_add(
                out=rowsum[:, 4:8], in0=rowsum[:, 4:8], scalar1=-E2)
            nc.scalar.activation(
                out=logden[:, 4:8], in_=rowsum[:, 4:8], func=AF.Ln)
            ldps = ps.tile([1, MT], F32, tag="ps")
            nc.tensor.matmul(ldps, lhsT=ones_col, rhs=logden, start=True, stop=True)
            pps = ps.tile([1, 1], F32, tag="ps")
            nc.tensor.matmul(pps, lhsT=ones_col, rhs=pacc, start=True, stop=True)

            l1 = big.tile([1, 1], F32, tag="l1")
            nc.vector.tensor_reduce(out=l1, in_=ldps, axis=AX.X, op=OP.add)
            res = big.tile([1, 1], F32, tag="res")
            # res = (l1 - 2*pps) / N2 in two fused ops
            nc.vector.scalar_tensor_tensor(
                out=res, in0=pps, scalar=-2.0, in1=l1,
                op0=OP.mult, op1=OP.add,
            )
            nc.vector.tensor_scalar_mul(out=res, in0=res, scalar1=1.0 / N2)
            nc.sync.dma_start(out=out_d[:, :], in_=res)

    nc.compile()
    return nc


_NC = None


def kernel(proj_1: np.ndarray, proj_2: np.ndarray) -> np.ndarray:
    global _NC, LAST_RESULT
    import os

    reps = np.concatenate(
        [np.asarray(proj_1, np.float32), np.asarray(proj_2, np.float32)], axis=0
    )
    assert reps.shape == (N2, D)

    in_maps = [
        {"xn": np.ascontiguousarray(np.roll(reps, -c * RPC, axis=0)).astype(ml_dtypes.bfloat16)}
        for c in range(NCORES)
    ]

    if _NC is None:
        _NC = _build_nc()

    trace = bool(os.environ.get("CONTRASTIVE_TRACE"))
    result = run_bass_kernel_spmd(
        _NC, in_maps, core_ids=list(range(NCORES)), trace=trace
    )
    LAST_RESULT = result
    total = sum(float(r["out"][0, 0]) for r in result.results)
    return np.float32(total)



# revision 16
# speedup vs baseline: 1.0333x; 1.0033x over previous
"""Contrastive (NT-Xent) loss kernel for 8 Trainium2 NeuronCores.

Math (reference): z = l2norm(concat(proj_1, proj_2)) [8192,128];
sim = z @ z.T; loss = mean_i( log(sum_{j!=i} exp(2*sim_ij)) - 2*pos_i ).

Sharding: rows of the 8192x8192 sim matrix are split 1024/core. Each core
receives the full rep matrix rotated by core*1024 rows (host-side layout
only), so its own rows are always local columns [0,1024) and the positive
partners are at [4096,5120) -- one identical SPMD program, static offsets.
Each core emits one partial scalar; the host sums 8 floats.

Device pipeline (per core), interleaved in groups of 2048 columns so the
ACT engine (the bottleneck: exp at 1 elem/cycle/lane) starts ~5us in:
  group g: DMA 4 natural bf16 chunks -> row norms (DVE, fp32 accum) ->
  1/norm via ln/exp (compact [128,16], ACT) -> per-row scale (DVE) ->
  PE-transpose into normalized bf16 X^T columns -> main quarter g:
  8 m-tiles x (4 bf16 matmuls -> fused exp+row-sum ACT op, [128,2048]
  PSUM). bf16 input halves the DMA head; loss rel err stays ~1e-6.
Then log-denominator, positives dot, partition-sum matmuls, one scalar out.
"""

import ml_dtypes
import numpy as np

import concourse.bass as bass
import concourse.tile as tile
from concourse import bacc, mybir
from concourse.bass_utils import run_bass_kernel_spmd
from concourse.hw_specs import get_activation_tables
from concourse.masks import make_identity

B = 4096
D = 128
N2 = 2 * B            # 8192 total rows
NCORES = 8
RPC = N2 // NCORES    # 1024 rows per core
MT = RPC // 128       # 8 m-tiles of 128 rows
NCH = N2 // 512       # 16 column chunks of 512
NG = 4                # groups of 4 chunks (2048 cols)
TEMP = 0.5
E2 = float(np.exp(1.0 / TEMP))   # exp(sim_ii / T) with sim_ii == 1

F32 = mybir.dt.float32
F32R = mybir.dt.float32r
BF16 = mybir.dt.bfloat16
AX = mybir.AxisListType
OP = mybir.AluOpType
AF = mybir.ActivationFunctionType

LAST_RESULT = None  # BassKernelResults of the most recent run (for test.py)


def _build_nc():
    nc = bacc.Bacc("TRN2", target_bir_lowering=False)
    xn_d = nc.declare_dram_parameter("xn", [N2, D], BF16, isOutput=False)
    out_d = nc.declare_dram_parameter("out", [1, 1], F32, isOutput=True)

    # Pre-place the one ACT table set that covers both Ln and Exp, so the
    # greedy per-func chooser never inserts mid-kernel table switches.
    table_names = list(get_activation_tables(nc.m.arch).keys())
    combined_id = table_names.index("natural_log_exp_and_others")

    with tile.TileContext(nc) as tc:
        with (
            tc.tile_pool(name="big", bufs=1) as big,
            tc.tile_pool(name="work", bufs=3) as work,
            tc.tile_pool(name="scr", bufs=2) as scr,
            tc.tile_pool(name="ps", bufs=2, space="PSUM") as ps,
        ):
            nc.scalar.add_instruction(mybir.InstLoadActFuncSet(
                name=nc.get_next_instruction_name(), ins=[], outs=[],
                act_func_set_id=combined_id))

            xn_all = big.tile([128, 64, 128], BF16, tag="xn")  # [p, j, d]: row j*128+p
            xhat = big.tile([D, N2], BF16, tag="xhat")         # normalized reps^T
            ns_c = big.tile([128, 64], F32, tag="ns")         # |row|^2 compact
            lnn = big.tile([128, 64], F32, tag="lnn")
            s_c = big.tile([128, 64], F32, tag="s")           # 1/|row| compact
            ones_col = big.tile([128, 1], F32, tag="ones_col")
            rs_all = big.tile([128, MT * NG], F32, tag="rs")  # exp row-sums (m, g)
            ident = big.tile([128, 128], BF16, tag="ident")
            pacc = big.tile([128, 1], F32, tag="pacc")

            def prep_group(g):
                """DMA 4 natural chunks, row norms, 1/norm, scale, transpose
                into xhat columns [2048g, 2048(g+1))."""
                for c in range(4 * g, 4 * g + 4):
                    # alternate the two HWDGE queues (SP / ACT)
                    eng = nc.sync if c % 2 == 0 else nc.scalar
                    eng.dma_start(
                        out=xn_all[:, c * 4:(c + 1) * 4, :],
                        in_=xn_d[c * 512:(c + 1) * 512, :].rearrange(
                            "(t p) d -> p t d", p=128
                        ),
                    )
                    # normsq per 128-row block: fused square + row-sum
                    for j in range(4):
                        jj = c * 4 + j
                        sqs = work.tile([128, 128], F32, tag="sqs")
                        blk = xn_all[:, jj, :]
                        nc.vector.scalar_tensor_tensor(
                            out=sqs, in0=blk, scalar=1.0, in1=blk,
                            op0=OP.mult, op1=OP.mult,
                            accum_out=ns_c[:, jj:jj + 1],
                        )
                # 1/norm = exp(-0.5*ln(normsq)); same ACT table set as exp.
                # group 0 is the latency-critical head: do it per chunk-pair
                # so the chain doesn't wait for all 4 chunk DMAs.
                subs = 2 if g == 0 else 1
                # high priority: these tiny ops must not queue behind the
                # previous quarter's exp stream on ACT (they gate this
                # group's scale->transpose chain and its PSUM slot release)
                with tc.high_priority():
                    for i in range(subs):
                        w = 16 // subs
                        gsl = slice(16 * g + i * w, 16 * g + (i + 1) * w)
                        nc.scalar.activation(
                            out=lnn[:, gsl], in_=ns_c[:, gsl], func=AF.Ln
                        )
                        nc.scalar.activation(
                            out=s_c[:, gsl], in_=lnn[:, gsl], func=AF.Exp,
                            scale=-0.5,
                        )
                # scale rows, PE-transpose into xhat columns (bf16)
                tp = ps.tile([128, 2048], BF16, tag="ps")
                for c in range(4 * g, 4 * g + 4):
                    xsc = work.tile([128, 4, 128], BF16, tag="xsc")
                    nc.vector.tensor_mul(
                        xsc,
                        xn_all[:, c * 4:(c + 1) * 4, :],
                        s_c[:, c * 4:(c + 1) * 4].broadcast_to([128, 4, 128]),
                    )
                    for j in range(4):
                        nc.tensor.transpose(
                            tp[:, (c % 4) * 512 + j * 128:(c % 4) * 512 + (j + 1) * 128],
                            xsc[:, j, :],
                            ident[:],
                        )
                    nc.vector.tensor_copy(
                        xhat[:, c * 512:(c + 1) * 512],
                        tp[:, (c % 4) * 512:(c % 4 + 1) * 512],
                    )
                if g == 2:
                    # positives dot (needs xhat chunks 0,1 and 8,9)
                    prod = scr.tile([128, RPC], F32, tag="scr")
                    nc.vector.scalar_tensor_tensor(
                        out=prod,
                        in0=xhat[:, 0:RPC],
                        scalar=1.0,
                        in1=xhat[:, B:B + RPC],
                        op0=OP.mult,
                        op1=OP.mult,
                        accum_out=pacc,
                    )

            def quarter_half(g, half):
                """4 m-tiles of main work on columns [2048g, 2048(g+1))."""
                for m in range(4 * half, 4 * half + 4):
                    pst = ps.tile([128, 2048], F32, tag="ps")
                    lhsT = xhat[:, m * 128:(m + 1) * 128]
                    for s4 in range(4):
                        col = g * 2048 + s4 * 512
                        nc.tensor.matmul(
                            pst[:, s4 * 512:(s4 + 1) * 512],
                            lhsT=lhsT,
                            rhs=xhat[:, col:col + 512],
                            start=True,
                            stop=True,
                        )
                    sc = scr.tile([128, 2048], BF16, tag="scr")
                    nc.scalar.activation(
                        out=sc,
                        in_=pst,
                        func=AF.Exp,
                        scale=1.0 / TEMP,
                        accum_out=rs_all[:, m * NG + g:m * NG + g + 1],
                    )

            nc.vector.memset(ones_col, 1.0)
            make_identity(nc, ident[:])

            # interleave: group g+1 prep emitted mid-quarter-g so its DMAs,
            # DVE work and PSUM slot use hide under the ACT exp stream
            prep_group(0)
            quarter_half(0, 0)
            prep_group(1)
            quarter_half(0, 1)
            quarter_half(1, 0)
            prep_group(2)
            quarter_half(1, 1)
            quarter_half(2, 0)
            prep_group(3)
            quarter_half(2, 1)
            quarter_half(3, 0)
            quarter_half(3, 1)

            # ---- finals ----
            rowsum = big.tile([128, MT], F32, tag="rowsum")
            nc.vector.tensor_reduce(
                out=rowsum,
                in_=rs_all[:].rearrange("p (m g) -> p m g", g=NG),
                axis=AX.X,
                op=OP.add,
            )
            den = big.tile([128, MT], F32, tag="den")
            nc.vector.tensor_scalar_add(out=den, in0=rowsum, scalar1=-E2)
            logden = big.tile([128, MT], F32, tag="logden")
            nc.scalar.activation(out=logden, in_=den, func=AF.Ln)
            ldps = ps.tile([1, MT], F32, tag="ps")
            nc.tensor.matmul(ldps, lhsT=ones_col, rhs=logden, start=True, stop=True)
            pps = ps.tile([1, 1], F32, tag="ps")
            nc.tensor.matmul(pps, lhsT=ones_col, rhs=pacc, start=True, stop=True)

            l1 = big.tile([1, 1], F32, tag="l1")
            nc.vector.tensor_reduce(out=l1, in_=ldps, axis=AX.X, op=OP.add)
            t2 = big.tile([1, 1], F32, tag="t2")
            nc.vector.tensor_scalar_mul(out=t2, in0=pps, scalar1=-2.0)
            res = big.tile([1, 1], F32, tag="res")
            nc.vector.tensor_add(res, l1, t2)
            nc.vector.tensor_scalar_mul(out=res, in0=res, scalar1=1.0 / N2)
            nc.sync.dma_start(out=out_d[:, :], in_=res)

    nc.compile()
    return nc


_NC = None


def kernel(proj_1: np.ndarray, proj_2: np.ndarray) -> np.ndarray:
    global _NC, LAST_RESULT
    import os

    reps = np.concatenate(
        [np.asarray(proj_1, np.float32), np.asarray(proj_2, np.float32)], axis=0
    )
    assert reps.shape == (N2, D)

    in_maps = [
        {"xn": np.ascontiguousarray(np.roll(reps, -c * RPC, axis=0)).astype(ml_dtypes.bfloat16)}
        for c in range(NCORES)
    ]

    if _NC is None:
        _NC = _build_nc()

    trace = bool(os.environ.get("CONTRASTIVE_TRACE"))
    result = run_bass_kernel_spmd(
        _NC, in_maps, core_ids=list(range(NCORES)), trace=trace
    )
    LAST_RESULT = result
    total = sum(float(r["out"][0, 0]) for r in result.results)
    return np.float32(total)

